# revision 16
# baseline (speedup 1.0000x reference)
"""Trainium2 Bass kernel for 3-NN inverse-distance-weighted feature interpolation.

Problem: B=4, N=65536, S=512, C=3, D=1, K_NN=3.
  dists[b,n,s] = ||xyz1[b,:,n] - xyz2[b,:,s]||^2
  top-3 smallest per (b,n); w_k = (1/(d_k+eps)) / sum_j; out = sigmoid(2 * sum w_k f2[idx_k])

Sharding: 8 cores; core c handles batch c//2, query half c%2 (32768 queries).

Per-core pipeline (256 tiles of 128 queries):
  PE:  P[128,512] = 2*dot - sq2 computed as a K=21 bf16 matmul whose rows are
       an exact multi-split (hi/mid/lo bf16) of the fp32 operands — every
       partial product is exact in fp32, giving ~fp32 accuracy at bf16 speed.
  DVE: max8 (top-8 of P = 3 smallest distances), max_index (uint16 indices)
  GPSIMD: indirect_copy gather of f2 at the top-3 indices (per-16-partition
          shared-column semantics; wanted values land on a mod-16 "diagonal",
          extracted with a precomputed E mask)
  tail (batched over 8 tiles): d_k = sq1 - m_k (+eps); r = 1/d; out = sigmoid(2*num/den)
  output staged [128,128], PE-transposed, DMA'd contiguously.

Host-side prep (cheap O(N) numpy): bf16 splits of the matmul operands,
sq1 (+eps) per query in a DMA-friendly transposed layout.
"""
import sys
sys.path.insert(0, '/opt/trn_rl_repo')

import numpy as np
import ml_dtypes
from contextlib import ExitStack

import concourse.bass as bass
import concourse.bacc as bacc
import concourse.tile as tile
from concourse import mybir
from concourse.bass_utils import run_bass_kernel_spmd

F32 = mybir.dt.float32
BF16 = mybir.dt.bfloat16
U16 = mybir.dt.uint16
I32 = mybir.dt.int32
AX = mybir.AxisListType
OP = mybir.AluOpType
ACTF = mybir.ActivationFunctionType

B, N, S = 4, 65536, 512
N_CORES = 8
NQ = N // 2              # queries per core
TQ = 128                 # queries per tile
NT = NQ // TQ            # 256 tiles per core
GRP = 16                 # tiles per tail group
EPS = 1e-8
KR = 21                  # split-matmul contraction rows

_nc_cache = {}
TRACE = False


def build_nc(n_tiles=NT):
    nc = bacc.Bacc("TRN2", target_bir_lowering=False, debug=False,
                   num_devices=N_CORES)
    lhs_d = nc.dram_tensor("lhs_d", [KR, NQ], BF16, kind="ExternalInput").ap()
    rhs_d = nc.dram_tensor("rhs_d", [KR, S], BF16, kind="ExternalInput").ap()
    f2c = nc.dram_tensor("f2c", [1, S], F32, kind="ExternalInput").ap()
    sq1_d = nc.dram_tensor("sq1_d", [128, NT], F32, kind="ExternalInput").ap()
    outc = nc.dram_tensor("outc", [NQ], F32, kind="ExternalOutput").ap()
    out2d = outc.rearrange("(t p) -> t p", p=TQ)   # [256,128]

    with tile.TileContext(nc) as tc, ExitStack() as ctx:
        const = ctx.enter_context(tc.tile_pool(name="const", bufs=1))
        setup = ctx.enter_context(tc.tile_pool(name="setup", bufs=1))
        lt_pool = ctx.enter_context(tc.tile_pool(name="lt", bufs=3))
        psb_pool = ctx.enter_context(tc.tile_pool(name="psb", bufs=4))
        ps_P = ctx.enter_context(tc.tile_pool(name="psP", bufs=6, space="PSUM"))
        ps_T = ctx.enter_context(tc.tile_pool(name="psT", bufs=1, space="PSUM"))
        grp_pool = ctx.enter_context(tc.tile_pool(name="grp", bufs=4))
        tail_pool = ctx.enter_context(tc.tile_pool(name="tail", bufs=2))
        stage_pool = ctx.enter_context(tc.tile_pool(name="stage", bufs=2))

        # ---------- constants ----------
        # E[p, i] = 1.0 iff (i % 16) == (p % 16), shape [128, 48]
        ramp = const.tile([128, 48], I32)
        nc.gpsimd.iota(ramp[:], pattern=[[0, 3], [1, 16]], base=0,
                       channel_multiplier=0)
        pid = const.tile([128, 48], I32)
        nc.gpsimd.iota(pid[:], pattern=[[0, 48]], base=0, channel_multiplier=1)
        pmod = const.tile([128, 48], I32)
        nc.vector.tensor_scalar(pmod[:], pid[:], 15, None, op0=OP.bitwise_and)
        E = const.tile([128, 48], F32)
        nc.vector.tensor_tensor(E[:], ramp[:], pmod[:], op=OP.is_equal)

        # identity for PE transpose
        iot_p = const.tile([128, 128], I32)
        nc.gpsimd.iota(iot_p[:], pattern=[[0, 128]], base=0, channel_multiplier=1)
        iot_f = const.tile([128, 128], I32)
        nc.gpsimd.iota(iot_f[:], pattern=[[1, 128]], base=0, channel_multiplier=0)
        ident = const.tile([128, 128], F32)
        nc.vector.tensor_tensor(ident[:], iot_p[:], iot_f[:], op=OP.is_equal)

        # ---------- per-core setup ----------
        f2b = setup.tile([128, S], F32)
        nc.sync.dma_start(f2b[:], f2c[0:1, :].partition_broadcast(128))
        rhs = setup.tile([KR, S], BF16)
        nc.sync.dma_start(rhs[:], rhs_d[:])
        sq1_sb = setup.tile([128, NT], F32)
        nc.sync.dma_start(sq1_sb[:], sq1_d[:])

        # ---------- main loop ----------
        n_grp = n_tiles // GRP
        stage = None
        for g in range(n_grp):
            m8g = grp_pool.tile([128, 8 * GRP], F32, tag="m8g")
            mig = grp_pool.tile([128, 8 * GRP], U16, tag="mig")
            g48g = grp_pool.tile([128, 48 * GRP], F32, tag="g48g")
            if g % (128 // GRP) == 0:
                stage = stage_pool.tile([128, 128], F32, tag="stage")
                if n_tiles % 128 != 0:
                    nc.vector.memset(stage[:], 0.0)  # test-only partial blocks

            lt = lt_pool.tile([KR, TQ * GRP], BF16)
            nc.sync.dma_start(lt[:], lhs_d[:, g * GRP * TQ:(g + 1) * GRP * TQ])

            for j in range(GRP):
                # distance matmul: P = 2*dot - sq2 (exact bf16 splits)
                pP = ps_P.tile([TQ, S], F32)
                nc.tensor.matmul(pP[:], lt[:, j * TQ:(j + 1) * TQ], rhs[:],
                                 start=True, stop=True)

                # top-8 values + indices (PSUM reads keep the shared
                # DVE/GpSimd SBUF port free for the gathers)
                nc.vector.max(m8g[:, 8 * j:8 * j + 8], pP[:])
                nc.vector.max_index(mig[:, 8 * j:8 * j + 8],
                                    m8g[:, 8 * j:8 * j + 8], pP[:])

            for j in range(GRP):
                # f2 gather (shared-column; diagonal extraction later)
                nc.gpsimd.indirect_copy(g48g[:, 48 * j:48 * j + 48], f2b[:],
                                        mig[:, 8 * j:8 * j + 3],
                                        i_know_ap_gather_is_preferred=True)

            # ---------- batched tail for the group ----------
            m3 = m8g[:].rearrange("p (j e) -> p j e", e=8)[:, :, 0:3]
            sq1r = sq1_sb[:, g * GRP:(g + 1) * GRP].unsqueeze(-1) \
                                                   .broadcast_to([128, GRP, 3])
            d3 = tail_pool.tile([128, 3 * GRP], F32, tag="d3")
            d3v = d3[:].rearrange("p (j e) -> p j e", e=3)
            nc.vector.tensor_tensor(d3v, sq1r, m3, op=OP.subtract)

            r = tail_pool.tile([128, 3 * GRP], F32, tag="r")
            nc.vector.reciprocal(r[:], d3[:])
            den = tail_pool.tile([128, GRP], F32, tag="den")
            nc.vector.reduce_sum(den[:], r[:].rearrange("p (j e) -> p j e", e=3),
                                 axis=AX.X)

            r_rep = r[:].rearrange("p (j e) -> p j e", e=3).unsqueeze(-1) \
                        .broadcast_to([128, GRP, 3, 16])
            g4 = g48g[:].rearrange("p (j k q) -> p j k q", k=3, q=16)
            t1 = tail_pool.tile([128, 48 * GRP], F32, tag="t1")
            t1v = t1[:].rearrange("p (j k q) -> p j k q", k=3, q=16)
            nc.vector.tensor_tensor(t1v, g4, r_rep, op=OP.mult)

            e_rep = E[:].unsqueeze(1).broadcast_to([128, GRP, 48])
            t2 = tail_pool.tile([128, 48 * GRP], F32, tag="t2")
            t2v = t2[:].rearrange("p (j i) -> p j i", i=48)
            nc.vector.tensor_tensor(t2v, t1[:].rearrange("p (j i) -> p j i", i=48),
                                    e_rep, op=OP.mult)
            num = tail_pool.tile([128, GRP], F32, tag="num")
            nc.vector.reduce_sum(num[:], t2v, axis=AX.X)

            rden = tail_pool.tile([128, GRP], F32, tag="rden")
            nc.vector.reciprocal(rden[:], den[:])
            outv = tail_pool.tile([128, GRP], F32, tag="outv")
            nc.vector.tensor_tensor(outv[:], num[:], rden[:], op=OP.mult)

            # sigmoid(2x) == (tanh(x)+1)/2 ; write into the stage block
            col = (g * GRP) % 128
            nc.scalar.activation(stage[:, col:col + GRP], outv[:], ACTF.Sigmoid,
                                 scale=2.0)

            # ---------- flush a filled (or final partial) stage block ----------
            if (g + 1) % (128 // GRP) == 0 or g == n_grp - 1:
                blk = (g * GRP) // 128
                filled = (g * GRP) % 128 + GRP
                pT = ps_T.tile([128, 128], F32)
                nc.tensor.transpose(pT[:], stage[:], ident[:])
                oT = stage_pool.tile([128, 128], F32, tag="oT")
                nc.scalar.copy(oT[:], pT[:])
                nc.sync.dma_start(out2d[blk * 128:blk * 128 + filled, :],
                                  oT[0:filled, :])

    nc.compile()
    return nc


def _get_nc():
    if "nc" not in _nc_cache:
        _nc_cache["nc"] = build_nc()
    return _nc_cache["nc"]


def _split3(v32):
    """Exact-ish 3-way bf16 split of fp32 array: v ~= h + m + l."""
    h = v32.astype(ml_dtypes.bfloat16)
    r = (v32 - h.astype(np.float32)).astype(np.float32)
    m = r.astype(ml_dtypes.bfloat16)
    l = (r - m.astype(np.float32)).astype(ml_dtypes.bfloat16)
    return h, m, l


def make_core_inputs(xyz1h, xyz2b, f2b_):
    """Build one core's input map. xyz1h [3, NQ] f32, xyz2b [3, S], f2b_ [1, S]."""
    xh, xm, xl = {}, {}, {}
    yh, ym, yl = {}, {}, {}
    for c in range(3):
        xh[c], xm[c], xl[c] = _split3(xyz1h[c])
        yh[c], ym[c], yl[c] = _split3((2.0 * xyz2b[c]).astype(np.float32))
    x2 = xyz2b.astype(np.float32)
    sq2 = ((x2[0] * x2[0] + x2[1] * x2[1]) + x2[2] * x2[2]).astype(np.float32)
    sh, sm, sl = _split3(-sq2)

    onesq = np.ones(NQ, ml_dtypes.bfloat16)
    lhs_rows, rhs_rows = [], []
    # magnitude-ordered terms: hh + sq2h, then first-order, then second-order
    for c in range(3):
        lhs_rows.append(xh[c]); rhs_rows.append(yh[c])
    lhs_rows.append(onesq); rhs_rows.append(sh)
    for c in range(3):
        lhs_rows.append(xh[c]); rhs_rows.append(ym[c])
        lhs_rows.append(xm[c]); rhs_rows.append(yh[c])
    lhs_rows.append(onesq); rhs_rows.append(sm)
    for c in range(3):
        lhs_rows.append(xh[c]); rhs_rows.append(yl[c])
        lhs_rows.append(xl[c]); rhs_rows.append(yh[c])
        lhs_rows.append(xm[c]); rhs_rows.append(ym[c])
    lhs_rows.append(onesq); rhs_rows.append(sl)
    assert len(lhs_rows) == KR
    lhs = np.stack(lhs_rows).astype(ml_dtypes.bfloat16)
    rhs = np.stack(rhs_rows).astype(ml_dtypes.bfloat16)

    x1 = xyz1h.astype(np.float32)
    sq1 = ((x1[0] * x1[0] + x1[1] * x1[1]) + x1[2] * x1[2]).astype(np.float32)
    sq1e = (sq1 + np.float32(EPS)).astype(np.float32)
    # transposed layout: sq1_t[p, t] = sq1e[t*128 + p]
    sq1_t = np.ascontiguousarray(sq1e.reshape(NT, TQ).T)

    return {
        "lhs_d": np.ascontiguousarray(lhs),
        "rhs_d": np.ascontiguousarray(rhs),
        "f2c": np.ascontiguousarray(f2b_.astype(np.float32)),
        "sq1_d": sq1_t,
    }


def kernel(xyz1, xyz2, points2):
    xyz1 = np.ascontiguousarray(np.asarray(xyz1, dtype=np.float32))
    xyz2 = np.ascontiguousarray(np.asarray(xyz2, dtype=np.float32))
    points2 = np.ascontiguousarray(np.asarray(points2, dtype=np.float32))

    nc = _get_nc()
    in_maps = []
    for c in range(N_CORES):
        b, h = c // 2, c % 2
        in_maps.append(make_core_inputs(xyz1[b][:, h * NQ:(h + 1) * NQ],
                                        xyz2[b], points2[b]))
    res = run_bass_kernel_spmd(nc, in_maps, core_ids=list(range(N_CORES)),
                               trace=TRACE)
    if TRACE:
        _nc_cache["last_exec_time_ns"] = res.exec_time_ns
        _nc_cache["last_results"] = res
    out = np.empty((B, N), dtype=np.float32)
    for c in range(N_CORES):
        b, h = c // 2, c % 2
        out[b, h * NQ:(h + 1) * NQ] = res.results[c]["outc"]
    return out


if __name__ == "__main__":
    rng = np.random.default_rng(0)
    xyz1 = rng.standard_normal((B, 3, N)).astype(np.float32)
    xyz2 = rng.standard_normal((B, 3, S)).astype(np.float32)
    points2 = rng.standard_normal((B, 1, S)).astype(np.float32)
    out = kernel(xyz1, xyz2, points2)
    print("out", out.shape, out.dtype, out[0, :5])


# revision 17
# speedup vs baseline: 1.1667x; 1.1667x over previous
"""Trainium2 Bass kernel for 3-NN inverse-distance-weighted feature interpolation.

Problem: B=4, N=65536, S=512, C=3, D=1, K_NN=3.
  dists[b,n,s] = ||xyz1[b,:,n] - xyz2[b,:,s]||^2
  top-3 smallest per (b,n); w_k = (1/(d_k+eps)) / sum_j; out = sigmoid(2 * sum w_k f2[idx_k])

Sharding: 8 cores; core c handles batch c//2, query half c%2 (32768 queries).

Per-core pipeline (256 tiles of 128 queries):
  PE:  P[128,512] = 2*dot - sq2 computed as a K=21 bf16 matmul whose rows are
       an exact multi-split (hi/mid/lo bf16) of the fp32 operands — every
       partial product is exact in fp32, giving ~fp32 accuracy at bf16 speed.
  DVE: max8 (top-8 of P = 3 smallest distances), max_index (uint16 indices)
  GPSIMD: indirect_copy gather of f2 at the top-3 indices (per-16-partition
          shared-column semantics; wanted values land on a mod-16 "diagonal",
          extracted with a precomputed E mask)
  tail (batched over 8 tiles): d_k = sq1 - m_k (+eps); r = 1/d; out = sigmoid(2*num/den)
  output staged [128,128], PE-transposed, DMA'd contiguously.

Host-side prep (cheap O(N) numpy): bf16 splits of the matmul operands,
sq1 (+eps) per query in a DMA-friendly transposed layout.
"""
import sys
sys.path.insert(0, '/opt/trn_rl_repo')

import numpy as np
import ml_dtypes
from contextlib import ExitStack

import concourse.bass as bass
import concourse.bacc as bacc
import concourse.tile as tile
from concourse import mybir
from concourse.bass_utils import run_bass_kernel_spmd

F32 = mybir.dt.float32
BF16 = mybir.dt.bfloat16
U16 = mybir.dt.uint16
I32 = mybir.dt.int32
AX = mybir.AxisListType
OP = mybir.AluOpType
ACTF = mybir.ActivationFunctionType

B, N, S = 4, 65536, 512
N_CORES = 8
NQ = N // 2              # queries per core
TQ = 128                 # queries per tile
NT = NQ // TQ            # 256 tiles per core
GRP = 8                  # tiles per tail group
EPS = 1e-8
KR = 21                  # split-matmul contraction rows

_nc_cache = {}
TRACE = False


def build_nc(n_tiles=NT):
    nc = bacc.Bacc("TRN2", target_bir_lowering=False, debug=False,
                   num_devices=N_CORES)
    lhs_d = nc.dram_tensor("lhs_d", [KR, NQ], BF16, kind="ExternalInput").ap()
    rhs_d = nc.dram_tensor("rhs_d", [KR, S], BF16, kind="ExternalInput").ap()
    f2c = nc.dram_tensor("f2c", [1, S], F32, kind="ExternalInput").ap()
    sq1_d = nc.dram_tensor("sq1_d", [128, NT], F32, kind="ExternalInput").ap()
    outc = nc.dram_tensor("outc", [NQ], F32, kind="ExternalOutput").ap()
    out2d = outc.rearrange("(t p) -> t p", p=TQ)   # [256,128]

    with tile.TileContext(nc) as tc, ExitStack() as ctx:
        const = ctx.enter_context(tc.tile_pool(name="const", bufs=1))
        setup = ctx.enter_context(tc.tile_pool(name="setup", bufs=1))
        lt_pool = ctx.enter_context(tc.tile_pool(name="lt", bufs=3))
        psb_pool = ctx.enter_context(tc.tile_pool(name="psb", bufs=4))
        ps_P = ctx.enter_context(tc.tile_pool(name="psP", bufs=6, space="PSUM"))
        ps_T = ctx.enter_context(tc.tile_pool(name="psT", bufs=1, space="PSUM"))
        grp_pool = ctx.enter_context(tc.tile_pool(name="grp", bufs=4))
        tail_pool = ctx.enter_context(tc.tile_pool(name="tail", bufs=2))
        stage_pool = ctx.enter_context(tc.tile_pool(name="stage", bufs=2))

        # ---------- constants ----------
        # E[p, i] = 1.0 iff (i % 16) == (p % 16), shape [128, 48]
        ramp = const.tile([128, 48], I32)
        nc.gpsimd.iota(ramp[:], pattern=[[0, 3], [1, 16]], base=0,
                       channel_multiplier=0)
        pid = const.tile([128, 48], I32)
        nc.gpsimd.iota(pid[:], pattern=[[0, 48]], base=0, channel_multiplier=1)
        pmod = const.tile([128, 48], I32)
        nc.vector.tensor_scalar(pmod[:], pid[:], 15, None, op0=OP.bitwise_and)
        E = const.tile([128, 48], F32)
        nc.vector.tensor_tensor(E[:], ramp[:], pmod[:], op=OP.is_equal)

        # identity for PE transpose
        iot_p = const.tile([128, 128], I32)
        nc.gpsimd.iota(iot_p[:], pattern=[[0, 128]], base=0, channel_multiplier=1)
        iot_f = const.tile([128, 128], I32)
        nc.gpsimd.iota(iot_f[:], pattern=[[1, 128]], base=0, channel_multiplier=0)
        ident = const.tile([128, 128], F32)
        nc.vector.tensor_tensor(ident[:], iot_p[:], iot_f[:], op=OP.is_equal)

        # ---------- per-core setup ----------
        f2b = setup.tile([128, S], F32)
        nc.sync.dma_start(f2b[:], f2c[0:1, :].partition_broadcast(128))
        rhs = setup.tile([KR, S], BF16)
        nc.sync.dma_start(rhs[:], rhs_d[:])
        sq1_sb = setup.tile([128, NT], F32)
        nc.sync.dma_start(sq1_sb[:], sq1_d[:])

        # ---------- main loop ----------
        n_grp = n_tiles // GRP
        stage = None
        for g in range(n_grp):
            m8g = grp_pool.tile([128, 8 * GRP], F32, tag="m8g")
            mig = grp_pool.tile([128, 8 * GRP], U16, tag="mig")
            g48g = grp_pool.tile([128, 48 * GRP], F32, tag="g48g")
            if g % (128 // GRP) == 0:
                stage = stage_pool.tile([128, 128], F32, tag="stage")
                if n_tiles % 128 != 0:
                    nc.vector.memset(stage[:], 0.0)  # test-only partial blocks

            lt = lt_pool.tile([KR, TQ * GRP], BF16)
            nc.sync.dma_start(lt[:], lhs_d[:, g * GRP * TQ:(g + 1) * GRP * TQ])

            for j in range(GRP):
                # distance matmul: P = 2*dot - sq2 (exact bf16 splits)
                pP = ps_P.tile([TQ, S], F32)
                nc.tensor.matmul(pP[:], lt[:, j * TQ:(j + 1) * TQ], rhs[:],
                                 start=True, stop=True)

                # top-8 values + indices (PSUM reads keep the shared
                # DVE/GpSimd SBUF port free for the gathers)
                nc.vector.max(m8g[:, 8 * j:8 * j + 8], pP[:])
                nc.vector.max_index(mig[:, 8 * j:8 * j + 8],
                                    m8g[:, 8 * j:8 * j + 8], pP[:])

            for j in range(GRP):
                # f2 gather (shared-column; diagonal extraction later)
                nc.gpsimd.indirect_copy(g48g[:, 48 * j:48 * j + 48], f2b[:],
                                        mig[:, 8 * j:8 * j + 3],
                                        i_know_ap_gather_is_preferred=True)

            # ---------- batched tail for the group ----------
            m3 = m8g[:].rearrange("p (j e) -> p j e", e=8)[:, :, 0:3]
            sq1r = sq1_sb[:, g * GRP:(g + 1) * GRP].unsqueeze(-1) \
                                                   .broadcast_to([128, GRP, 3])
            d3 = tail_pool.tile([128, 3 * GRP], F32, tag="d3")
            d3v = d3[:].rearrange("p (j e) -> p j e", e=3)
            nc.vector.tensor_tensor(d3v, sq1r, m3, op=OP.subtract)

            r = tail_pool.tile([128, 3 * GRP], F32, tag="r")
            nc.vector.reciprocal(r[:], d3[:])
            den = tail_pool.tile([128, GRP], F32, tag="den")
            nc.vector.reduce_sum(den[:], r[:].rearrange("p (j e) -> p j e", e=3),
                                 axis=AX.X)

            r_rep = r[:].rearrange("p (j e) -> p j e", e=3).unsqueeze(-1) \
                        .broadcast_to([128, GRP, 3, 16])
            g4 = g48g[:].rearrange("p (j k q) -> p j k q", k=3, q=16)
            t1 = tail_pool.tile([128, 48 * GRP], F32, tag="t1")
            t1v = t1[:].rearrange("p (j k q) -> p j k q", k=3, q=16)
            nc.vector.tensor_tensor(t1v, g4, r_rep, op=OP.mult)

            e_rep = E[:].unsqueeze(1).broadcast_to([128, GRP, 48])
            t2 = tail_pool.tile([128, 48 * GRP], F32, tag="t2")
            t2v = t2[:].rearrange("p (j i) -> p j i", i=48)
            nc.vector.tensor_tensor(t2v, t1[:].rearrange("p (j i) -> p j i", i=48),
                                    e_rep, op=OP.mult)
            num = tail_pool.tile([128, GRP], F32, tag="num")
            nc.vector.reduce_sum(num[:], t2v, axis=AX.X)

            rden = tail_pool.tile([128, GRP], F32, tag="rden")
            nc.vector.reciprocal(rden[:], den[:])
            outv = tail_pool.tile([128, GRP], F32, tag="outv")
            nc.vector.tensor_tensor(outv[:], num[:], rden[:], op=OP.mult)

            # sigmoid(2x) == (tanh(x)+1)/2 ; write into the stage block
            col = (g * GRP) % 128
            nc.scalar.activation(stage[:, col:col + GRP], outv[:], ACTF.Sigmoid,
                                 scale=2.0)

            # ---------- flush a filled (or final partial) stage block ----------
            if (g + 1) % (128 // GRP) == 0 or g == n_grp - 1:
                blk = (g * GRP) // 128
                filled = (g * GRP) % 128 + GRP
                pT = ps_T.tile([128, 128], F32)
                nc.tensor.transpose(pT[:], stage[:], ident[:])
                oT = stage_pool.tile([128, 128], F32, tag="oT")
                nc.scalar.copy(oT[:], pT[:])
                nc.sync.dma_start(out2d[blk * 128:blk * 128 + filled, :],
                                  oT[0:filled, :])

    nc.compile()
    return nc


def _get_nc():
    if "nc" not in _nc_cache:
        _nc_cache["nc"] = build_nc()
    return _nc_cache["nc"]


def _split3(v32):
    """Exact-ish 3-way bf16 split of fp32 array: v ~= h + m + l."""
    h = v32.astype(ml_dtypes.bfloat16)
    r = (v32 - h.astype(np.float32)).astype(np.float32)
    m = r.astype(ml_dtypes.bfloat16)
    l = (r - m.astype(np.float32)).astype(ml_dtypes.bfloat16)
    return h, m, l


def make_core_inputs(xyz1h, xyz2b, f2b_):
    """Build one core's input map. xyz1h [3, NQ] f32, xyz2b [3, S], f2b_ [1, S]."""
    xh, xm, xl = {}, {}, {}
    yh, ym, yl = {}, {}, {}
    for c in range(3):
        xh[c], xm[c], xl[c] = _split3(xyz1h[c])
        yh[c], ym[c], yl[c] = _split3((2.0 * xyz2b[c]).astype(np.float32))
    x2 = xyz2b.astype(np.float32)
    sq2 = ((x2[0] * x2[0] + x2[1] * x2[1]) + x2[2] * x2[2]).astype(np.float32)
    sh, sm, sl = _split3(-sq2)

    onesq = np.ones(NQ, ml_dtypes.bfloat16)
    lhs_rows, rhs_rows = [], []
    # magnitude-ordered terms: hh + sq2h, then first-order, then second-order
    for c in range(3):
        lhs_rows.append(xh[c]); rhs_rows.append(yh[c])
    lhs_rows.append(onesq); rhs_rows.append(sh)
    for c in range(3):
        lhs_rows.append(xh[c]); rhs_rows.append(ym[c])
        lhs_rows.append(xm[c]); rhs_rows.append(yh[c])
    lhs_rows.append(onesq); rhs_rows.append(sm)
    for c in range(3):
        lhs_rows.append(xh[c]); rhs_rows.append(yl[c])
        lhs_rows.append(xl[c]); rhs_rows.append(yh[c])
        lhs_rows.append(xm[c]); rhs_rows.append(ym[c])
    lhs_rows.append(onesq); rhs_rows.append(sl)
    assert len(lhs_rows) == KR
    lhs = np.stack(lhs_rows).astype(ml_dtypes.bfloat16)
    rhs = np.stack(rhs_rows).astype(ml_dtypes.bfloat16)

    x1 = xyz1h.astype(np.float32)
    sq1 = ((x1[0] * x1[0] + x1[1] * x1[1]) + x1[2] * x1[2]).astype(np.float32)
    sq1e = (sq1 + np.float32(EPS)).astype(np.float32)
    # transposed layout: sq1_t[p, t] = sq1e[t*128 + p]
    sq1_t = np.ascontiguousarray(sq1e.reshape(NT, TQ).T)

    return {
        "lhs_d": np.ascontiguousarray(lhs),
        "rhs_d": np.ascontiguousarray(rhs),
        "f2c": np.ascontiguousarray(f2b_.astype(np.float32)),
        "sq1_d": sq1_t,
    }


def kernel(xyz1, xyz2, points2):
    xyz1 = np.ascontiguousarray(np.asarray(xyz1, dtype=np.float32))
    xyz2 = np.ascontiguousarray(np.asarray(xyz2, dtype=np.float32))
    points2 = np.ascontiguousarray(np.asarray(points2, dtype=np.float32))

    nc = _get_nc()
    in_maps = []
    for c in range(N_CORES):
        b, h = c // 2, c % 2
        in_maps.append(make_core_inputs(xyz1[b][:, h * NQ:(h + 1) * NQ],
                                        xyz2[b], points2[b]))
    res = run_bass_kernel_spmd(nc, in_maps, core_ids=list(range(N_CORES)),
                               trace=TRACE)
    if TRACE:
        _nc_cache["last_exec_time_ns"] = res.exec_time_ns
        _nc_cache["last_results"] = res
    out = np.empty((B, N), dtype=np.float32)
    for c in range(N_CORES):
        b, h = c // 2, c % 2
        out[b, h * NQ:(h + 1) * NQ] = res.results[c]["outc"]
    return out


if __name__ == "__main__":
    rng = np.random.default_rng(0)
    xyz1 = rng.standard_normal((B, 3, N)).astype(np.float32)
    xyz2 = rng.standard_normal((B, 3, S)).astype(np.float32)
    points2 = rng.standard_normal((B, 1, S)).astype(np.float32)
    out = kernel(xyz1, xyz2, points2)
    print("out", out.shape, out.dtype, out[0, :5])


# revision 19
# speedup vs baseline: 1.2103x; 1.0374x over previous
"""Trainium2 Bass kernel for 3-NN inverse-distance-weighted feature interpolation.

Problem: B=4, N=65536, S=512, C=3, D=1, K_NN=3.
  dists[b,n,s] = ||xyz1[b,:,n] - xyz2[b,:,s]||^2
  top-3 smallest per (b,n); w_k = (1/(d_k+eps)) / sum_j; out = sigmoid(2 * sum w_k f2[idx_k])

Sharding: 8 cores; core c handles batch c//2, query half c%2 (32768 queries).

Per-core pipeline (256 tiles of 128 queries):
  PE:  P[128,512] = 2*dot - sq2 computed as a K=21 bf16 matmul whose rows are
       an exact multi-split (hi/mid/lo bf16) of the fp32 operands — every
       partial product is exact in fp32, giving ~fp32 accuracy at bf16 speed.
  DVE: max8 (top-8 of P = 3 smallest distances), max_index (uint16 indices)
  GPSIMD: indirect_copy gather of f2 at the top-3 indices (per-16-partition
          shared-column semantics; wanted values land on a mod-16 "diagonal",
          extracted with a precomputed E mask)
  tail (batched over 8 tiles): d_k = sq1 - m_k (+eps); r = 1/d; out = sigmoid(2*num/den)
  output staged [128,128], PE-transposed, DMA'd contiguously.

Host-side prep (cheap O(N) numpy): bf16 splits of the matmul operands,
sq1 (+eps) per query in a DMA-friendly transposed layout.
"""
import sys, os
sys.path.insert(0, '/opt/trn_rl_repo')

import numpy as np
import ml_dtypes
from contextlib import ExitStack

import concourse.bass as bass
import concourse.bacc as bacc
import concourse.tile as tile
from concourse import mybir
from concourse.bass_utils import run_bass_kernel_spmd

F32 = mybir.dt.float32
BF16 = mybir.dt.bfloat16
U16 = mybir.dt.uint16
I32 = mybir.dt.int32
AX = mybir.AxisListType
OP = mybir.AluOpType
ACTF = mybir.ActivationFunctionType

B, N, S = 4, 65536, 512
N_CORES = 8
NQ = N // 2              # queries per core
TQ = 128                 # queries per tile
NT = NQ // TQ            # 256 tiles per core
GRP = 8                  # tiles per tail group
EPS = 1e-8
KR = 21                  # split-matmul contraction rows

_nc_cache = {}
TRACE = False
ABLATE = os.environ.get("ABLATE", "")


def build_nc(n_tiles=NT):
    nc = bacc.Bacc("TRN2", target_bir_lowering=False, debug=False,
                   num_devices=N_CORES)
    lhs_d = nc.dram_tensor("lhs_d", [KR, NQ], BF16, kind="ExternalInput").ap()
    rhs_d = nc.dram_tensor("rhs_d", [KR, S], BF16, kind="ExternalInput").ap()
    f2c = nc.dram_tensor("f2c", [1, S], F32, kind="ExternalInput").ap()
    sq1_d = nc.dram_tensor("sq1_d", [128, NT], F32, kind="ExternalInput").ap()
    outc = nc.dram_tensor("outc", [NQ], F32, kind="ExternalOutput").ap()
    out2d = outc.rearrange("(t p) -> t p", p=TQ)   # [256,128]

    with tile.TileContext(nc) as tc, ExitStack() as ctx:
        const = ctx.enter_context(tc.tile_pool(name="const", bufs=1))
        setup = ctx.enter_context(tc.tile_pool(name="setup", bufs=1))
        lt_pool = ctx.enter_context(tc.tile_pool(name="lt", bufs=3))
        psb_pool = ctx.enter_context(tc.tile_pool(name="psb", bufs=4))
        ps_P = ctx.enter_context(tc.tile_pool(name="psP", bufs=6, space="PSUM"))
        ps_T = ctx.enter_context(tc.tile_pool(name="psT", bufs=1, space="PSUM"))
        grp_pool = ctx.enter_context(tc.tile_pool(name="grp", bufs=4))
        tail_pool = ctx.enter_context(tc.tile_pool(name="tail", bufs=2))
        stage_pool = ctx.enter_context(tc.tile_pool(name="stage", bufs=2))

        # ---------- constants ----------
        # E[p, i] = 1.0 iff (i % 16) == (p % 16), shape [128, 48]
        ramp = const.tile([128, 48], I32)
        nc.gpsimd.iota(ramp[:], pattern=[[0, 3], [1, 16]], base=0,
                       channel_multiplier=0)
        pid = const.tile([128, 48], I32)
        nc.gpsimd.iota(pid[:], pattern=[[0, 48]], base=0, channel_multiplier=1)
        pmod = const.tile([128, 48], I32)
        nc.vector.tensor_scalar(pmod[:], pid[:], 15, None, op0=OP.bitwise_and)
        E = const.tile([128, 48], F32)
        nc.vector.tensor_tensor(E[:], ramp[:], pmod[:], op=OP.is_equal)

        # identity for PE transpose
        iot_p = const.tile([128, 128], I32)
        nc.gpsimd.iota(iot_p[:], pattern=[[0, 128]], base=0, channel_multiplier=1)
        iot_f = const.tile([128, 128], I32)
        nc.gpsimd.iota(iot_f[:], pattern=[[1, 128]], base=0, channel_multiplier=0)
        ident = const.tile([128, 128], F32)
        nc.vector.tensor_tensor(ident[:], iot_p[:], iot_f[:], op=OP.is_equal)

        # ---------- per-core setup ----------
        f2b = setup.tile([128, S], F32)
        nc.sync.dma_start(f2b[:], f2c[0:1, :].partition_broadcast(128))
        rhs = setup.tile([KR, S], BF16)
        nc.sync.dma_start(rhs[:], rhs_d[:])
        sq1_sb = setup.tile([128, NT], F32)
        nc.sync.dma_start(sq1_sb[:], sq1_d[:])

        g48c = None
        if "G" in ABLATE:
            g48c = const.tile([128, 48 * GRP], F32)
            nc.vector.memset(g48c[:], 1.0)

        # ---------- main loop ----------
        n_grp = n_tiles // GRP
        stage = None
        for g in range(n_grp):
            m8g = grp_pool.tile([128, 8 * GRP], F32, tag="m8g")
            mig = grp_pool.tile([128, 8 * GRP], U16, tag="mig")
            g48g = g48c if "G" in ABLATE else grp_pool.tile([128, 48 * GRP], F32, tag="g48g")
            if g % (128 // GRP) == 0:
                stage = stage_pool.tile([128, 128], F32, tag="stage")
                if n_tiles % 128 != 0:
                    nc.vector.memset(stage[:], 0.0)  # test-only partial blocks

            lt = lt_pool.tile([KR, TQ * GRP], BF16)
            nc.sync.dma_start(lt[:], lhs_d[:, g * GRP * TQ:(g + 1) * GRP * TQ])

            for j in range(GRP):
                # distance matmul: P = 2*dot - sq2 (exact bf16 splits)
                pP = ps_P.tile([TQ, S], F32)
                nc.tensor.matmul(pP[:], lt[:, j * TQ:(j + 1) * TQ], rhs[:],
                                 start=True, stop=True)

                # top-8 values + indices (PSUM reads keep the shared
                # DVE/GpSimd SBUF port free for the gathers)
                nc.vector.max(m8g[:, 8 * j:8 * j + 8], pP[:])
                nc.vector.max_index(mig[:, 8 * j:8 * j + 8],
                                    m8g[:, 8 * j:8 * j + 8], pP[:])

            if "G" not in ABLATE:
                for j in range(GRP):
                    # f2 gather (shared-column; diagonal extraction later)
                    nc.gpsimd.indirect_copy(g48g[:, 48 * j:48 * j + 48], f2b[:],
                                            mig[:, 8 * j:8 * j + 3],
                                            i_know_ap_gather_is_preferred=True)

            # ---------- batched tail for the group ----------
            m3 = m8g[:].rearrange("p (j e) -> p j e", e=8)[:, :, 0:3]
            sq1r = sq1_sb[:, g * GRP:(g + 1) * GRP].unsqueeze(-1) \
                                                   .broadcast_to([128, GRP, 3])
            d3 = tail_pool.tile([128, 3 * GRP], F32, tag="d3")
            d3v = d3[:].rearrange("p (j e) -> p j e", e=3)
            nc.vector.tensor_tensor(d3v, sq1r, m3, op=OP.subtract)

            r = tail_pool.tile([128, 3 * GRP], F32, tag="r")
            nc.vector.reciprocal(r[:], d3[:])
            den = tail_pool.tile([128, GRP], F32, tag="den")
            nc.vector.reduce_sum(den[:], r[:].rearrange("p (j e) -> p j e", e=3),
                                 axis=AX.X)

            r_rep = r[:].rearrange("p (j e) -> p j e", e=3).unsqueeze(-1) \
                        .broadcast_to([128, GRP, 3, 16])
            g4 = g48g[:].rearrange("p (j k q) -> p j k q", k=3, q=16)
            t1 = tail_pool.tile([128, 48 * GRP], F32, tag="t1")
            t1v = t1[:].rearrange("p (j k q) -> p j k q", k=3, q=16)
            nc.vector.tensor_tensor(t1v, g4, r_rep, op=OP.mult)

            e_rep = E[:].unsqueeze(1).broadcast_to([128, GRP, 48])
            t2 = tail_pool.tile([128, 48 * GRP], F32, tag="t2")
            t2v = t2[:].rearrange("p (j i) -> p j i", i=48)
            nc.vector.tensor_tensor(t2v, t1[:].rearrange("p (j i) -> p j i", i=48),
                                    e_rep, op=OP.mult)
            num = tail_pool.tile([128, GRP], F32, tag="num")
            nc.vector.reduce_sum(num[:], t2v, axis=AX.X)

            rden = tail_pool.tile([128, GRP], F32, tag="rden")
            nc.vector.reciprocal(rden[:], den[:])
            outv = tail_pool.tile([128, GRP], F32, tag="outv")
            nc.vector.tensor_tensor(outv[:], num[:], rden[:], op=OP.mult)

            # sigmoid(2x) == (tanh(x)+1)/2 ; write into the stage block
            col = (g * GRP) % 128
            nc.scalar.activation(stage[:, col:col + GRP], outv[:], ACTF.Sigmoid,
                                 scale=2.0)

            # ---------- flush a filled (or final partial) stage block ----------
            if (g + 1) % (128 // GRP) == 0 or g == n_grp - 1:
                blk = (g * GRP) // 128
                filled = (g * GRP) % 128 + GRP
                pT = ps_T.tile([128, 128], F32)
                nc.tensor.transpose(pT[:], stage[:], ident[:])
                oT = stage_pool.tile([128, 128], F32, tag="oT")
                nc.scalar.copy(oT[:], pT[:])
                nc.sync.dma_start(out2d[blk * 128:blk * 128 + filled, :],
                                  oT[0:filled, :])

    nc.compile()
    return nc


def _get_nc():
    if "nc" not in _nc_cache:
        _nc_cache["nc"] = build_nc()
    return _nc_cache["nc"]


def _split3(v32):
    """Exact-ish 3-way bf16 split of fp32 array: v ~= h + m + l."""
    h = v32.astype(ml_dtypes.bfloat16)
    r = (v32 - h.astype(np.float32)).astype(np.float32)
    m = r.astype(ml_dtypes.bfloat16)
    l = (r - m.astype(np.float32)).astype(ml_dtypes.bfloat16)
    return h, m, l


def make_core_inputs(xyz1h, xyz2b, f2b_):
    """Build one core's input map. xyz1h [3, NQ] f32, xyz2b [3, S], f2b_ [1, S]."""
    xh, xm, xl = {}, {}, {}
    yh, ym, yl = {}, {}, {}
    for c in range(3):
        xh[c], xm[c], xl[c] = _split3(xyz1h[c])
        yh[c], ym[c], yl[c] = _split3((2.0 * xyz2b[c]).astype(np.float32))
    x2 = xyz2b.astype(np.float32)
    sq2 = ((x2[0] * x2[0] + x2[1] * x2[1]) + x2[2] * x2[2]).astype(np.float32)
    sh, sm, sl = _split3(-sq2)

    onesq = np.ones(NQ, ml_dtypes.bfloat16)
    lhs_rows, rhs_rows = [], []
    # magnitude-ordered terms: hh + sq2h, then first-order, then second-order
    for c in range(3):
        lhs_rows.append(xh[c]); rhs_rows.append(yh[c])
    lhs_rows.append(onesq); rhs_rows.append(sh)
    for c in range(3):
        lhs_rows.append(xh[c]); rhs_rows.append(ym[c])
        lhs_rows.append(xm[c]); rhs_rows.append(yh[c])
    lhs_rows.append(onesq); rhs_rows.append(sm)
    for c in range(3):
        lhs_rows.append(xh[c]); rhs_rows.append(yl[c])
        lhs_rows.append(xl[c]); rhs_rows.append(yh[c])
        lhs_rows.append(xm[c]); rhs_rows.append(ym[c])
    lhs_rows.append(onesq); rhs_rows.append(sl)
    assert len(lhs_rows) == KR
    lhs = np.stack(lhs_rows).astype(ml_dtypes.bfloat16)
    rhs = np.stack(rhs_rows).astype(ml_dtypes.bfloat16)

    x1 = xyz1h.astype(np.float32)
    sq1 = ((x1[0] * x1[0] + x1[1] * x1[1]) + x1[2] * x1[2]).astype(np.float32)
    sq1e = (sq1 + np.float32(EPS)).astype(np.float32)
    # transposed layout: sq1_t[p, t] = sq1e[t*128 + p]
    sq1_t = np.ascontiguousarray(sq1e.reshape(NT, TQ).T)

    return {
        "lhs_d": np.ascontiguousarray(lhs),
        "rhs_d": np.ascontiguousarray(rhs),
        "f2c": np.ascontiguousarray(f2b_.astype(np.float32)),
        "sq1_d": sq1_t,
    }


def kernel(xyz1, xyz2, points2):
    xyz1 = np.ascontiguousarray(np.asarray(xyz1, dtype=np.float32))
    xyz2 = np.ascontiguousarray(np.asarray(xyz2, dtype=np.float32))
    points2 = np.ascontiguousarray(np.asarray(points2, dtype=np.float32))

    nc = _get_nc()
    in_maps = []
    for c in range(N_CORES):
        b, h = c // 2, c % 2
        in_maps.append(make_core_inputs(xyz1[b][:, h * NQ:(h + 1) * NQ],
                                        xyz2[b], points2[b]))
    res = run_bass_kernel_spmd(nc, in_maps, core_ids=list(range(N_CORES)),
                               trace=TRACE)
    if TRACE:
        _nc_cache["last_exec_time_ns"] = res.exec_time_ns
        _nc_cache["last_results"] = res
    out = np.empty((B, N), dtype=np.float32)
    for c in range(N_CORES):
        b, h = c // 2, c % 2
        out[b, h * NQ:(h + 1) * NQ] = res.results[c]["outc"]
    return out


if __name__ == "__main__":
    rng = np.random.default_rng(0)
    xyz1 = rng.standard_normal((B, 3, N)).astype(np.float32)
    xyz2 = rng.standard_normal((B, 3, S)).astype(np.float32)
    points2 = rng.standard_normal((B, 1, S)).astype(np.float32)
    out = kernel(xyz1, xyz2, points2)
    print("out", out.shape, out.dtype, out[0, :5])
